# revision 54
# baseline (speedup 1.0000x reference)
"""Multi-head attention (B=4, L=2048, D=1024, H=16) on 8 Trainium2 NeuronCores.

Sharding: core c handles batch b=c//2 and head-half half=c%2 (8 heads = 512 of
the 1024 projection output dims).  Each core computes its heads' Q/K/V
projections, attention, and a full-L partial of the output projection
(contracting only its 512 head dims).  The host sums the two partials per batch
(the "all-reduce after fc" done at gather time).

v2 restructure vs baseline: the attention inner loop is ScalarE-bound (one
[128,1024] exp per lk tile, ~1.1us, vs ~0.64us of PE streaming).  All
projection matmuls (K/Q of the next pair, V inside the first set, output
projection per finished lq block) are statically interleaved into the
attention loop's tensor-idle slots, and the lk loop is software-pipelined
(S one iteration ahead of AV/dn) so ScalarE never starves.  PSUM: scores
2x[128,1024] + av/dn 3x[128,512] + proj 1x[128,512] = 8 banks.
"""

import sys

import numpy as np

if "/opt/trn_rl_repo" not in sys.path:
    sys.path.insert(0, "/opt/trn_rl_repo")

import concourse.bass as bass
import concourse.mybir as mybir
from concourse import bacc
import concourse.tile as tile
from concourse.bass import ts

F32 = mybir.dt.float32
F16 = mybir.dt.float16
I16 = mybir.dt.int16
EXP = mybir.ActivationFunctionType.Exp

L = 2048          # sequence length
D = 1024          # model dim
OC = 512          # output-projection dims owned by one core (8 heads x 64)
NPAIR = 4         # head pairs per core (pair = 128 projection dims)
NLQB = 4          # lq blocks of 512
LQB = 512
NLK = 16          # lk tiles of 128
ND = 8            # d-model tiles of 128
N_CORES = 8

# lk tiles whose exp runs on VectorE via the Schraudolph int16 trick
# (abandoned: HW error ~1.4e-2 at 2/16 tiles, too close to the 2e-2 gate)
DVE_EXP_LKS = frozenset()
# lk tiles whose pt accumulation runs on GpSimd (abandoned: 2.5us/add
# makes GpSimd the pacer)
GP_LKS = frozenset()
SCHRAUDOLPH_C1 = 1024.0 * 1.4426950408889634
SCHRAUDOLPH_C2 = (15.0 - 0.0595) * 1024.0


def build_program():
    nc = bacc.Bacc("TRN2", debug=False, enable_asserts=False,
                   target_bir_lowering=False)

    qT = nc.dram_tensor("qT", [D, L], F16, kind="ExternalInput").ap()
    kT = nc.dram_tensor("kT", [D, L], F16, kind="ExternalInput").ap()
    vT = nc.dram_tensor("vT", [D, L], F16, kind="ExternalInput").ap()
    wqT = nc.dram_tensor("wqT", [D, OC], F16, kind="ExternalInput").ap()
    wkT = nc.dram_tensor("wkT", [D, OC], F16, kind="ExternalInput").ap()
    wvT = nc.dram_tensor("wvT", [D, OC], F16, kind="ExternalInput").ap()
    woT = nc.dram_tensor("woT", [OC, D], F16, kind="ExternalInput").ap()
    out = nc.dram_tensor("out", [L, D], F32, kind="ExternalOutput").ap()

    with tile.TileContext(nc, pool_alloc_mode="queue") as tc:
        build_body(nc, tc, qT, kT, vT, wqT, wkT, wvT, woT, out)
    nc.compile()
    return nc


def build_body(nc, tc, qT, kT, vT, wqT, wkT, wvT, woT, out):
    # ---- PSUM pools: 2*2 + 2*1 + 2*1 = 8 banks --------------------------
    st_pool = tc.alloc_tile_pool(name="st", bufs=2, space="PSUM")
    ad_pool = tc.alloc_tile_pool(name="ad", bufs=2, space="PSUM")
    pj_pool = tc.alloc_tile_pool(name="pj", bufs=2, space="PSUM")

    # ---- SBUF pools -----------------------------------------------------
    pt_pool = tc.alloc_tile_pool(name="pt", bufs=3)
    rc_pool = tc.alloc_tile_pool(name="rc", bufs=2)
    oc_pool = tc.alloc_tile_pool(name="ocp", bufs=2)
    vs_pool = tc.alloc_tile_pool(name="vs", bufs=4)
    pa_pool = tc.alloc_tile_pool(name="pa", bufs=2)

    frees = []

    def single(shape, name):
        t, free = tc.tile(shape, F16, name=name)
        frees.append(free)
        return t

    ones_t = single([128, 64], "ones_t")
    nc.vector.memset(ones_t[:], 1.0)

    # HAM warm-up: ~3.5us of dummy matmuls during the initial DMA wait so
    # the PE clock-gate is already at 8/8 (2.4GHz) when real work arrives
    warm = pj_pool.tile([64, 64], F32, tag="pj", name="warm")
    for _ in range(45):
        nc.tensor.matmul(warm[:], ones_t[:], ones_t[:, 0:64],
                         start=True, stop=True)

    vh_t = single([128, NLK, OC], "vh_t")     # [l, lk, oc]
    ot_t = single([128, NPAIR, L], "ot_t")    # [o, pair, lq]
    kh_t = single([128, NPAIR, L], "kh_t")    # [oc_pair, pair, l]
    qh_t = single([128, NPAIR, L], "qh_t")

    wk = single([128, ND, OC], "wk")
    wq = single([128, ND, OC], "wq")
    wv = single([128, ND, OC], "wv")
    wo = single([128, NPAIR, D], "wo")
    kfull = single([128, ND, L], "kfull")
    qfull = single([128, ND, L], "qfull")

    # ---- input DMAs, chunked and ordered by first use ------------------
    # wk/wq arrive pair-by-pair so K(p0)/Q(p0) start after ~1.3MB, not 6MB
    wk_re = wkT.rearrange("(n p) o -> p n o", p=128)
    wq_re = wqT.rearrange("(n p) o -> p n o", p=128)
    k_re = kT.rearrange("(n p) l -> p n l", p=128)
    q_re = qT.rearrange("(n p) l -> p n l", p=128)
    # critical chain on the sync queue, in exact head-consumption order
    nc.sync.dma_start(out=wk[:, :, ts(0, 128)], in_=wk_re[:, :, ts(0, 128)])
    nc.sync.dma_start(out=kfull[:, :, ts(0, LQB)], in_=k_re[:, :, ts(0, LQB)])
    nc.sync.dma_start(out=kfull[:, :, ts(1, LQB)], in_=k_re[:, :, ts(1, LQB)])
    nc.sync.dma_start(out=wq[:, :, ts(0, 128)], in_=wq_re[:, :, ts(0, 128)])
    nc.sync.dma_start(out=kfull[:, :, ts(2, LQB)], in_=k_re[:, :, ts(2, LQB)])
    nc.sync.dma_start(out=kfull[:, :, ts(3, LQB)], in_=k_re[:, :, ts(3, LQB)])
    nc.sync.dma_start(out=qfull[:, :, ts(0, LQB)], in_=q_re[:, :, ts(0, LQB)])
    nc.sync.dma_start(out=qfull[:, :, ts(1, LQB)], in_=q_re[:, :, ts(1, LQB)])
    nc.sync.dma_start(out=wv[:], in_=wvT.rearrange("(n p) o -> p n o", p=128))
    for c in range(2, NLQB):
        nc.sync.dma_start(out=qfull[:, :, ts(c, LQB)],
                          in_=q_re[:, :, ts(c, LQB)])
    # later-pair weights + wo off the sync queue (SWDGE) -- needed only
    # tens of microseconds in
    for pp in range(1, NPAIR):
        nc.gpsimd.dma_start(out=wk[:, :, ts(pp, 128)],
                            in_=wk_re[:, :, ts(pp, 128)])
        nc.gpsimd.dma_start(out=wq[:, :, ts(pp, 128)],
                            in_=wq_re[:, :, ts(pp, 128)])
    nc.gpsimd.dma_start(out=wo[:], in_=woT.rearrange("(n p) m -> p n m",
                                                     p=128))

    vblks = {}

    def v_dma(lt):
        vblk = vs_pool.tile([128, ND, 128], F16, tag="vs", name=f"vblk{lt}")
        nc.sync.dma_start(
            out=vblk[:],
            in_=vT[:, ts(lt, 128)].rearrange("(n p) l -> p n l", p=128))
        vblks[lt] = vblk

    # ---- projection work as closures (one matmul per item) -------------
    def kq_items(w, src, p, blk2, dst, copy_eng):
        # two lq blocks (blk2, blk2+1) share each LDWEIGHTS of w[:, dt, p]
        st8 = {}

        def mk(dt, j):
            def f():
                if dt == 0 and j == 0:
                    st8[0] = pj_pool.tile([128, LQB], F32, tag="pj",
                                          name=f"kq{p}_{blk2}a")
                    st8[1] = pj_pool.tile([128, LQB], F32, tag="pj",
                                          name=f"kq{p}_{blk2}b")
                nc.tensor.matmul(st8[j][:], w[:, dt, ts(p, 128)],
                                 src[:, dt, ts(blk2 + j, LQB)],
                                 start=(dt == 0), stop=(dt == ND - 1))
                if dt == ND - 1:
                    copy_eng(dst[:, p, ts(blk2 + j, LQB)], st8[j][:])
            return f
        return [mk(dt, j) for dt in range(ND) for j in range(2)]

    def v_items(lt):
        st8 = {}

        def mk(dt):
            def f():
                if dt == 0:
                    if lt + 4 < NLK:
                        v_dma(lt + 4)
                    st8["ps"] = pj_pool.tile([128, OC], F32, tag="pj",
                                             name=f"vp{lt}")
                nc.tensor.matmul(st8["ps"][:], vblks[lt][:, dt, :],
                                 wv[:, dt, :],
                                 start=(dt == 0), stop=(dt == ND - 1))
                if dt == ND - 1:
                    nc.scalar.copy(vh_t[:, lt, :], st8["ps"][:])
                    del vblks[lt]
            return f
        return [mk(dt) for dt in range(ND)]

    def o_items(lt, copy_eng=None):
        # p-outer / mb-inner: one ot LDWEIGHTS serves both output halves
        st8 = {}
        cp = copy_eng or nc.vector.tensor_copy

        def mk(p, mb):
            def f():
                if p == 0 and mb == 0:
                    st8[0] = pj_pool.tile([128, 512], F32, tag="pj",
                                          name=f"op{lt}a")
                    st8[1] = pj_pool.tile([128, 512], F32, tag="pj",
                                          name=f"op{lt}b")
                    st8["oc"] = oc_pool.tile([128, D], F32, tag="oc",
                                             name=f"oc{lt}")
                nc.tensor.matmul(st8[mb][:], ot_t[:, p, ts(lt, 128)],
                                 wo[:, p, ts(mb, 512)],
                                 start=(p == 0), stop=(p == NPAIR - 1))
                if p == NPAIR - 1:
                    cp(st8["oc"][:, ts(mb, 512)], st8[mb][:])
                    if mb == 1:
                        nc.sync.dma_start(out=out[ts(lt, 128), :],
                                          in_=st8["oc"][:])
            return f
        return [mk(p, mb) for p in range(NPAIR) for mb in range(2)]

    # ---- flat attention pipeline over all (pair, lqb, lk) --------------
    # Software-pipelined one iteration ahead, including across set
    # boundaries, so ScalarE's exp stream never gaps.  The softmax
    # denominator is deferred: pt tiles accumulate into ptacc on VectorE
    # (fp16, 2x mode) and a single ones-matmul pair per set replaces the
    # per-lk denominator matmuls on the PE critical chain.
    iters = [(p, lqb, lk)
             for p in range(NPAIR) for lqb in range(NLQB)
             for lk in range(NLK)]
    cur = {}

    def emit_s(idx):
        p, lqb, lk = iters[idx]
        st = st_pool.tile([128, 1024], F32, tag="st",
                          name=f"st{p}_{lqb}_{lk}")
        nc.tensor.matmul(st[:, 0:512], kh_t[0:64, p, ts(lk, 128)],
                         qh_t[0:64, p, ts(lqb, LQB)], start=True, stop=True,
                         tile_position=(0, 0))
        nc.tensor.matmul(st[:, 512:1024], kh_t[64:128, p, ts(lk, 128)],
                         qh_t[64:128, p, ts(lqb, LQB)], start=True,
                         stop=True, tile_position=(64, 0))
        # pt tiles come in pairs [128, 2048] (two lk iterations side by
        # side) so the VectorE accumulation runs half as many, 2x-sized ops
        if lk % 2 == 0:
            ptp = pt_pool.tile([128, 2048], F16, tag="pt",
                               name=f"pt{p}_{lqb}_{lk}")
            cur[("ptp", idx)] = ptp
        else:
            ptp = cur[("ptp", idx - 1)]
        half = ptp[:, ts(lk % 2, 1024)]
        nc.scalar.activation(half, st[:], EXP)
        cur[idx] = ptp

    def emit_tail(idx):
        p, lqb, lk = iters[idx]
        ptp = cur.pop(idx)
        base = (lk % 2) * 1024
        if lk == 0:
            cur["av"] = ad_pool.tile([128, 512], F32, tag="ad",
                                     name=f"av{p}_{lqb}")
            cur["pae"] = pa_pool.tile([128, 2048], F16, tag="pa",
                                      name=f"pae{p}_{lqb}")
        av = cur["av"]
        # both halves start=True at lk 0: the has_written clear covers
        # each matmul's own partition range, so no memset is needed
        nc.tensor.matmul(av[0:64, :], vh_t[:, lk, ts(2 * p, 64)],
                         ptp[:, base:base + 512], start=(lk == 0),
                         stop=(lk == NLK - 1), tile_position=(0, 0),
                         skip_group_check=True)
        nc.tensor.matmul(av[64:128, :], vh_t[:, lk, ts(2 * p + 1, 64)],
                         ptp[:, base + 512:base + 1024], start=(lk == 0),
                         stop=(lk == NLK - 1), tile_position=(0, 64),
                         skip_group_check=True)
        if lk % 2 == 1:
            cur.pop(("ptp", idx - 1), None)
            if lk == 1:
                nc.vector.tensor_copy(cur["pae"][:], ptp[:])
            else:
                nc.vector.tensor_add(cur["pae"][:], cur["pae"][:], ptp[:])
        if lk == NLK - 1:
            pae = cur["pae"]

            # fold the two pair-halves on DVE so the PE only runs one
            # ones-matmul pair per set
            nc.vector.tensor_add(pae[:, 0:1024], pae[:, 0:1024],
                                 pae[:, 1024:2048])

            def finale():
                dn = ad_pool.tile([128, 512], F32, tag="ad",
                                  name=f"dn{p}_{lqb}")
                for h in range(2):
                    nc.tensor.matmul(dn[ts(h, 64), :], ones_t[:],
                                     pae[:, ts(h, 512)], start=True,
                                     stop=True, tile_position=(0, 64 * h),
                                     skip_group_check=True)
                rc = rc_pool.tile([128, 512], F32, tag="rc",
                                  name=f"rc{p}_{lqb}")
                nc.vector.reciprocal_approx_fast(out=rc[:], in_=dn[:])
                nc.vector.tensor_mul(ot_t[0:64, p, ts(lqb, LQB)],
                                     av[0:64, :], rc[0:64, :])
                nc.vector.tensor_mul(ot_t[64:128, p, ts(lqb, LQB)],
                                     av[64:128, :], rc[64:128, :])
            cur["finale"] = finale

    # ---- head: V chunk prefetch, K(p0) all blocks, Q(p0) blk 0+1 -------
    for lt in range(4):
        v_dma(lt)
    for fn in kq_items(wk, kfull, 0, 0, kh_t, nc.scalar.copy):
        fn()
    for fn in kq_items(wk, kfull, 0, 2, kh_t, nc.scalar.copy):
        fn()
    for fn in kq_items(wq, qfull, 0, 0, qh_t, nc.scalar.copy):
        fn()

    vcopy = nc.vector.tensor_copy

    # ---- per-set extra-work schedules ----------------------------------
    # kq group (pair, blk2) covers lq blocks blk2 and blk2+1 (16 items).
    def set_extras(p, lqb):
        extras = []
        if p == 0 and lqb == 0:
            for lt in range(NLK):
                extras += v_items(lt)
            extras += kq_items(wq, qfull, 0, 2, qh_t, vcopy)
            # K(p1) first half also fits here: set0 is tensor-bound, so
            # ScalarE idles regardless -- this keeps later p0 sets at
            # 1 item/slot
            extras += kq_items(wk, kfull, 1, 0, kh_t, vcopy)
            return extras, 10
        if p < 3:
            nxt = p + 1
            grps = [(wk, kfull, 0, kh_t), (wk, kfull, 2, kh_t),
                    (wq, qfull, 0, qh_t), (wq, qfull, 2, qh_t)]
            if p == 0:
                lo, hi = {1: (1, 2), 2: (2, 3), 3: (3, 4)}[lqb]
            else:
                lo, hi = {0: (0, 1), 1: (1, 2), 2: (2, 3), 3: (3, 4)}[lqb]
            for (w, src, blk2, dst) in grps[lo:hi]:
                extras += kq_items(w, src, nxt, blk2, dst, vcopy)
            return extras, 1
        if lqb >= 1:
            for lt in range(4 * (lqb - 1), 4 * lqb):
                extras += o_items(lt)
            return extras, 2
        return extras, 1

    # ---- run the flat pipeline (S stream 2 iterations ahead) -----------
    emit_s(0)
    emit_s(1)
    extras, per_slot = [], 1
    for idx in range(len(iters)):
        p, lqb, lk = iters[idx]
        if lk == 0:
            extras, per_slot = set_extras(p, lqb)
        if idx + 2 < len(iters):
            emit_s(idx + 2)
        if lk == 0 and "finale" in cur:
            cur.pop("finale")()
        n = len(extras) if lk == NLK - 1 else min(per_slot, len(extras))
        for fn in extras[:n]:
            fn()
        del extras[:n]
        emit_tail(idx)
    cur.pop("finale")()

    # ---- tail: output projection for the last lq block -----------------
    # copies alternate ScalarE/VectorE so consecutive groups never wait
    # on the same copy engine
    def tail_copy(dst, src):
        eng = nc.scalar.copy if tail_copy.flip else nc.vector.tensor_copy
        tail_copy.flip = not tail_copy.flip
        eng(dst, src)
    tail_copy.flip = True
    for lt in range(12, 16):
        for fn in o_items(lt, copy_eng=tail_copy):
            fn()

    for free in reversed(frees):
        free()
    for pool in (pa_pool, vs_pool, oc_pool, rc_pool, pt_pool, pj_pool,
                 ad_pool, st_pool):
        pool.release()


_CACHED_NC = None


def _get_program():
    global _CACHED_NC
    if _CACHED_NC is None:
        _CACHED_NC = build_program()
    return _CACHED_NC


def make_in_maps(q, k, v, w_q, w_k, w_v, w_o):
    in_maps = []
    for c in range(N_CORES):
        b, half = c // 2, c % 2
        osl = slice(half * OC, (half + 1) * OC)
        in_maps.append({
            "qT": np.ascontiguousarray(q[b].T).astype(np.float16),
            "kT": np.ascontiguousarray(k[b].T).astype(np.float16),
            "vT": np.ascontiguousarray(v[b].T).astype(np.float16),
            # temperature sqrt(d_k)=8 folded into the Q weights
            "wqT": np.ascontiguousarray(w_q[osl].T / 8.0).astype(np.float16),
            "wkT": np.ascontiguousarray(w_k[osl].T).astype(np.float16),
            "wvT": np.ascontiguousarray(w_v[osl].T).astype(np.float16),
            "woT": np.ascontiguousarray(w_o[:, osl].T).astype(np.float16),
        })
    return in_maps


def run_on_hw(q, k, v, w_q, w_k, w_v, w_o, trace=False, **trace_kwargs):
    from concourse.bass_utils import run_bass_kernel_spmd
    nc = _get_program()
    in_maps = make_in_maps(q, k, v, w_q, w_k, w_v, w_o)
    res = run_bass_kernel_spmd(nc, in_maps, core_ids=list(range(N_CORES)),
                               trace=trace, **trace_kwargs)
    B = 4
    outp = np.empty((B, L, D), np.float32)
    for b in range(B):
        outp[b] = res.results[2 * b]["out"] + res.results[2 * b + 1]["out"]
    return outp, res


def _numpy_fallback(q, k, v, w_q, w_k, w_v, w_o, mask):
    NEG = -1000000000.0
    B = q.shape[0]
    outs = []
    for b in range(B):
        qh = (q[b] @ w_q.T).reshape(L, 16, 64).transpose(1, 0, 2)
        kh = (k[b] @ w_k.T).reshape(L, 16, 64).transpose(1, 0, 2)
        vh = (v[b] @ w_v.T).reshape(L, 16, 64).transpose(1, 0, 2)
        s = np.einsum("hqd,hkd->hqk", qh / 8.0, kh)
        s = np.where(mask[b][None] == 0, NEG, s)
        s = s - s.max(axis=-1, keepdims=True)
        p = np.exp(s)
        p /= p.sum(axis=-1, keepdims=True)
        o = np.einsum("hqk,hkd->hqd", p, vh)
        o = o.transpose(1, 0, 2).reshape(L, D)
        outs.append(o @ w_o.T)
    return np.stack(outs).astype(np.float32)


def kernel(q, k, v, w_q, w_k, w_v, w_o, mask):
    q = np.asarray(q, np.float32)
    k = np.asarray(k, np.float32)
    v = np.asarray(v, np.float32)
    w_q = np.asarray(w_q, np.float32)
    w_k = np.asarray(w_k, np.float32)
    w_v = np.asarray(w_v, np.float32)
    w_o = np.asarray(w_o, np.float32)
    mask = np.asarray(mask)
    if not np.all(mask != 0):
        # never hit with the spec'd all-ones mask; correctness fallback
        return _numpy_fallback(q, k, v, w_q, w_k, w_v, w_o, mask)
    outp, _ = run_on_hw(q, k, v, w_q, w_k, w_v, w_o)
    return outp
